# revision 28
# baseline (speedup 1.0000x reference)
"""BertSelfAttention Trainium2 Bass kernel (fully-overlapped pipeline).

Problem: S=2048, B=4, H=1024, NH=16, DH=64, fp32.
  q/k/v = hidden @ W{q,k,v}.T + b   -> softmax((q k^T)/8 + mask) @ v

Sharding over 8 cores: batch (4) x head-group (2 groups of 8 heads).
Each core gets x=[2048,1024] (its batch), W shards [512,1024] (its 8
heads), mask [2048], and produces outT=[512,2048] (feature-major) which
the host transposes and scatters into the full [S,B,H] output.

The kernel is PE-bound (~314us of bf16 matmul at 2.4GHz is the hard
floor for scores[64-deep contraction] + PV + QKV projections +
transposes). Everything else is arranged to keep the PE streaming:

  - exp is SPLIT between ScalarE (ACT Exp) and the DVE: DVE tiles use a
    Schraudolph bit-trick exp -- one tensor_scalar (x*a+b) written as
    int16 and bitcast to bf16 gives 2^(x*log2e) with ~3% max elem err
    (the linear-mantissa wiggle; softmax normalization cancels the
    common mode). This keeps ScalarE (1.06us/tile, was saturated and
    accumulating lag at the 1.0us/iter steady pace) off the critical
    path. Error measured vs fp32 reference: ~1.4e-2 worst-case with all
    tiles on DVE, ~8e-3 with the 60/40 split here (budget 2e-2).
  - prologue: wq0/wk0 DMA on the sync queue in parallel with x0-3 on
    the gpsimd queue; kt g0/sg0 is produced as four 128-key minis so
    scores(0) fires right after qt0s0; first ACT ~13us (was 27.5us).
  - a flat 256-iteration stream emits EXP(t), scores(t+4), PV(t-8); two
    generators interleave chain/transpose production (gen-A: x packs +
    K/Q chains gating scores; gen-B: Wv packs + V chains gating PV).
  - PV accumulates [1+64, 512] per head with a leading ones-row (the
    softmax denominator lands in PSUM partition 0 for free); epilogue:
    DVE copy + reciprocal_approx_fast, then partition_broadcast and the
    multiply on GPSIMD (idle engine), DMA out feature-major; fins pop
    every iteration so the tail is just the last block's chain.
  - x0-3 staged fp32 (DMA-latency critical path, cast on the PSUM->SBUF
    copy after the PE transpose); x4-15 staged bf16 via cast DMA;
    Wq/Wk groups 1-3 go through the XBAR dma_start_transpose off the PE.
"""

import numpy as np

import concourse.bass as bass
import concourse.mybir as mybir
import concourse.tile as tile
from concourse import bacc
from concourse.bass_utils import run_bass_kernel_spmd
from concourse.masks import make_identity

F32 = mybir.dt.float32
BF16 = mybir.dt.bfloat16
I16 = mybir.dt.int16
AF = mybir.ActivationFunctionType
ALU = mybir.AluOpType

S, B, H, NH, DH = 2048, 4, 1024, 16, 64
N_CORES = 8
HPC = 8            # heads per core
DPC = HPC * DH     # 512 output features per core
SC = S // 128      # 16 s-chunks
FC = H // 128      # 8 feature chunks
QG = S // 512      # 4 query groups
KC = S // 128      # 16 key chunks
NG = 4             # head-pair groups per core
LAG = 8            # PV trails EXP by this many tiles
AHEAD = 4          # scores are emitted this many tiles ahead of EXP

LOG2E = 1.4426950408889634
SCHRA_A = LOG2E * 128.0 / 8.0          # x * a : folds the 1/sqrt(DH)
SCHRA_TAU = 0.5 - 128.0 * 0.0430       # truncation + interp centering
SCHRA_B = 127.0 * 128.0 + SCHRA_TAU    # + mask*log2e*128 per-partition


def _dve_tile(t):
    """Which exp tiles run on the DVE (Schraudolph) vs ScalarE (ACT).

    Early blocks are chain-heavy on the DVE (pack copies, bias adds), so
    keep most exps on ScalarE there; later blocks alternate evenly."""
    bi, kc = divmod(t, KC)
    if bi < 6:
        return kc in (5, 11)
    if bi < 10:
        return kc % 3 == 2
    return kc % 2 == 1


def _emit(ctx, tc, nc, x, mask, wq, bq, wk, bk, wv, bv, outT):
    const_p = ctx.enter_context(tc.tile_pool(name="const", bufs=1))
    xstage_p = ctx.enter_context(tc.tile_pool(name="xstage", bufs=8))
    xstgb_p = ctx.enter_context(tc.tile_pool(name="xstgb", bufs=8))
    wstage_p = ctx.enter_context(tc.tile_pool(name="wstage", bufs=2))
    wstgb_p = ctx.enter_context(tc.tile_pool(name="wstgb", bufs=6))
    xt_p = ctx.enter_context(tc.tile_pool(name="xt", bufs=1))
    wvt_p = ctx.enter_context(tc.tile_pool(name="wvt", bufs=1))
    wt_p = ctx.enter_context(tc.tile_pool(name="wt", bufs=8))
    v_p = ctx.enter_context(tc.tile_pool(name="v", bufs=SC))
    qkt_p = ctx.enter_context(tc.tile_pool(name="qkt", bufs=4))
    ex_p = ctx.enter_context(tc.tile_pool(name="ex", bufs=10))
    ctxs_p = ctx.enter_context(tc.tile_pool(name="ctxs", bufs=2))
    rec_p = ctx.enter_context(tc.tile_pool(name="rec", bufs=1))
    bcs_p = ctx.enter_context(tc.tile_pool(name="bcs", bufs=2))
    outt_p = ctx.enter_context(tc.tile_pool(name="outt", bufs=3))

    # psum (8 banks): mm 2x2 (score tiles) + ctx 2x1 (PV accumulators /
    # prologue packs+chains) + qa 1 (gen-A) + qb 1 (gen-B)
    psum_mm = ctx.enter_context(tc.tile_pool(name="psmm", bufs=2, space="PSUM"))
    psum_ctx = ctx.enter_context(tc.tile_pool(name="psctx", bufs=2, space="PSUM"))
    psum_qa = ctx.enter_context(tc.tile_pool(name="psqa", bufs=1, space="PSUM"))
    psum_qb = ctx.enter_context(tc.tile_pool(name="psqb", bufs=1, space="PSUM"))

    # ---- constants ----
    # contiguous natural-layout loads (few big descriptors, land in ~1us)
    # FIRST on the sync queue, then the big wq0/wk0 staging reads which
    # gate the first qt/kt chains. (partition-major strided loads would
    # flood the queue with 4-byte descriptors and stall it for ~20us --
    # transpose on the PE instead.)
    mask_nat = const_p.tile([KC, 128], F32)
    nc.sync.dma_start(out=mask_nat, in_=mask.rearrange("(c p) -> c p", p=128))
    bq_nat = const_p.tile([NG, 128], F32)
    nc.sync.dma_start(out=bq_nat, in_=bq.rearrange("(g p) -> g p", p=128))
    bk_nat = const_p.tile([NG, 128], F32)
    nc.sync.dma_start(out=bk_nat, in_=bk.rearrange("(g p) -> g p", p=128))
    # v bias broadcast across partitions via a DMA broadcast read (a
    # gpsimd partition_broadcast would trigger a ~10us ucode library
    # load that head-of-line blocks the gpsimd queue at stream start)
    bv_bc = const_p.tile([128, DPC], F32)
    nc.sync.dma_start(out=bv_bc,
                      in_=bv.rearrange("(a f) -> a f", a=1)
                            .to_broadcast((128, DPC)))

    nat_q0 = wstage_p.tile([128, H], F32, tag="ws", name="wn_q0")
    nat_k0 = wstage_p.tile([128, H], F32, tag="ws", name="wn_k0")
    nc.sync.dma_start(out=nat_k0, in_=wk[0:128, :])
    nc.sync.dma_start(out=nat_q0, in_=wq[0:128, :])
    w_nat = {("q", 0): nat_q0, ("k", 0): nat_k0}

    ident = const_p.tile([128, 128], F32)
    make_identity(nc, ident)
    ident_bf = const_p.tile([128, 128], BF16)
    nc.vector.tensor_copy(ident_bf, ident)

    ones_col_f = const_p.tile([128, HPC, 1], F32)
    nc.vector.memset(ones_col_f, 1.0)
    # partition-major mask/bias tiles, filled by PE transpose in prologue
    mask_sb = const_p.tile([128, KC], F32)
    mask_dve = const_p.tile([128, KC], F32)
    bq_sb = const_p.tile([128, NG], F32)
    bk_sb = const_p.tile([128, NG], F32)

    # ---- staging (priority order) ----
    xt = xt_p.tile([128, FC, S], BF16)
    wvt = wvt_p.tile([128, FC, DPC], BF16)
    wqts = [wt_p.tile([128, FC, 128], BF16, tag="wt", name=f"wqt{g}")
            for g in range(NG)]
    wkts = [wt_p.tile([128, FC, 128], BF16, tag="wt", name=f"wkt{g}")
            for g in range(NG)]

    # x staged fp32 (cast DMAs run ~2x slower per transfer and only the
    # gpsimd queue can cast -- fp32 lets the feed spread across three
    # DMA queues so every chunk lands within ~25us of kernel start, the
    # timing the tile scheduler's sim assumes); 16 distinct buffers so
    # no DMA waits on PE buffer recycling. x12-15 are bf16 cast-DMAs on
    # the gpsimd queue (needed latest, halves their SBUF).
    def _x_tile(sc):
        if sc < 4 or 8 <= sc < 12:
            return xstage_p.tile([128, H], F32, tag="xs", name=f"xn{sc}")
        return xstgb_p.tile([128, H], BF16, tag="xb", name=f"xn{sc}")
    x_nat = [_x_tile(sc) for sc in range(SC)]
    # wv staged bf16 (not latency-critical, halves SBUF)
    wvn_p = ctx.enter_context(tc.tile_pool(name="wvn", bufs=4))
    for dc in range(4):
        w_nat[("v", dc)] = wvn_p.tile([128, H], BF16, tag="wv",
                                      name=f"wn_v{dc}")
    # bf16 staging for the XBAR-transposed late W groups
    for g in range(1, NG):
        for key in ("q", "k"):
            nat = wstgb_p.tile([128, H], BF16, tag="wb", name=f"wb_{key}{g}")
            w_nat[(key, g)] = nat

    def x_dma(sc):
        q = nc.gpsimd if (sc < 8 or sc >= 12) else nc.scalar
        q.dma_start(out=x_nat[sc], in_=x[sc * 128:(sc + 1) * 128, :])

    for sc in range(SC):
        x_dma(sc)
    for dc in range(4):
        nc.gpsimd.dma_start(out=w_nat[("v", dc)],
                            in_=wv[dc * 128:(dc + 1) * 128, :])
    # dummy partition_broadcast: pulls the gpsimd ucode library load
    # (~10us, blocks the gpsimd queue) into the prologue DMA dead-time
    # instead of the first epilogue fin
    pbc_warm = const_p.tile([2, 1], F32)
    nc.gpsimd.partition_broadcast(pbc_warm, ones_col_f[0:1, 0, :])
    # Wq/Wk g1-3 staging casts must ride the gpsimd queue (only queue
    # that can cast); emitted from inside the attention loop, after the
    # upfront x/wv feed has been fully dispatched.

    def emit_wg_late(g):
        for key, wsrc in (("q", wq), ("k", wk)):
            nat = w_nat[(key, g)]
            nc.gpsimd.dma_start(out=nat, in_=wsrc[g * 128:(g + 1) * 128, :])
        for key, dst in (("q", wqts[g]), ("k", wkts[g])):
            nat = w_nat[(key, g)]
            for fc in range(FC):
                nc.sync.dma_start_transpose(dst[:, fc, :],
                                            nat[:, fc * 128:(fc + 1) * 128])

    # ---- PE transpose packs (fp32 or bf16 in, bf16 out via the copy) ----
    def tp_pack(dst_view, src_nat, fc0, pool, tag):
        bf = src_nat.dtype == BF16
        pt = pool.tile([128, 4, 128], BF16 if bf else F32, tag=tag, name="pt")
        for j in range(4):
            fc = fc0 + j
            nc.tensor.transpose(pt[:, j, :],
                                src_nat[:, fc * 128:(fc + 1) * 128],
                                ident_bf if bf else ident)
            yield
        nc.vector.tensor_copy(dst_view, pt)
        yield

    def tp_x(sc, pool, tag):
        for fc0 in (0, 4):
            yield from tp_pack(xt[:, fc0:fc0 + 4, sc * 128:(sc + 1) * 128],
                               x_nat[sc], fc0, pool, tag)

    def tp_wqk(key, g, pool, tag):
        dst = wqts[g] if key == "q" else wkts[g]
        for fc0 in (0, 4):
            yield from tp_pack(dst[:, fc0:fc0 + 4, :], w_nat[(key, g)],
                               fc0, pool, tag)

    def tp_wv(dc, pool, tag):
        for fc0 in (0, 4):
            yield from tp_pack(wvt[:, fc0:fc0 + 4, dc * 128:(dc + 1) * 128],
                               w_nat[("v", dc)], fc0, pool, tag)

    # ---- projection chains ----
    # v_sb layout: [:, h, 0] = ones (denominator row), [:, h, 1:65] = V
    v_sb = [v_p.tile([128, HPC, DH + 1], BF16, tag="v", name=f"v{sc}")
            for sc in range(SC)]
    qts = {}
    kts = {}

    def get_qkt(kind, g):
        d = qts if kind == "qt" else kts
        if g not in d:
            d[g] = qkt_p.tile([128, S], BF16, tag="qkt", name=f"{kind}{g}")
        return d[g]

    def v_chain(sc, pool, tag):
        vp = pool.tile([128, DPC], F32, tag=tag, name=f"vp{sc}")
        for fc in range(FC):
            nc.tensor.matmul(vp, xt[:, fc, sc * 128:(sc + 1) * 128],
                             wvt[:, fc, :], start=(fc == 0),
                             stop=(fc == FC - 1))
            yield
        nc.gpsimd.tensor_copy(v_sb[sc][:, :, 0:1], ones_col_f)
        # bias-add + bf16 cast (DVE: gpsimd cannot read PSUM)
        nc.vector.tensor_add(v_sb[sc][:, :, 1:DH + 1],
                             vp.rearrange("p (h d) -> p h d", d=DH),
                             bv_bc.rearrange("p (h d) -> p h d", d=DH))
        yield

    def qk_chain(kind, g, sg, pool, tag):
        bias_sb = bq_sb if kind == "qt" else bk_sb
        wt_src = wqts[g] if kind == "qt" else wkts[g]
        qk_dst = get_qkt(kind, g)
        ssl = slice(sg * 512, (sg + 1) * 512)
        qp = pool.tile([128, 512], F32, tag=tag, name=f"{kind}{g}s{sg}p")
        for fc in range(FC):
            nc.tensor.matmul(qp, wt_src[:, fc, :], xt[:, fc, ssl],
                             start=(fc == 0), stop=(fc == FC - 1))
            yield
        nc.vector.tensor_scalar_add(qk_dst[:, ssl], qp, bias_sb[:, g:g + 1])
        yield

    def kt_mini(kc, pool, tag):
        # 128-key chunk of the g0 kt chain: lets scores(kc) fire before
        # the full 512-key chain exists (prologue latency)
        qk_dst = get_qkt("kt", 0)
        ssl = slice(kc * 128, (kc + 1) * 128)
        qp = pool.tile([128, 128], F32, tag=tag, name=f"ktm{kc}")
        for fc in range(FC):
            nc.tensor.matmul(qp, wkts[0][:, fc, :], xt[:, fc, ssl],
                             start=(fc == 0), stop=(fc == FC - 1))
        nc.vector.tensor_scalar_add(qk_dst[:, ssl], qp, bk_sb[:, 0:1])

    done = set()

    def run_now(gen_):
        for _ in gen_:
            pass

    # ---- prologue: minimum to start the exp stream ----
    # tp_x0 -> mask/bias transposes -> wk0 pack -> kt minis interleaved
    # with tp_x1-3 -> wq0 pack -> qt0s0; then scores(0-3) fire together.
    run_now(tp_x(0, psum_ctx, "ctx"))
    # transpose the contiguous mask/bias loads into partition-major form
    for src_nat, dst, w in ((mask_nat, mask_sb, KC), (bq_nat, bq_sb, NG),
                            (bk_nat, bk_sb, NG)):
        ps = psum_ctx.tile([128, w], F32, tag="ctx", name="cst")
        nc.tensor.transpose(ps, src_nat, ident[0:w, 0:w])
        nc.vector.tensor_copy(dst, ps)
    # Schraudolph per-partition offset: mask*log2e*128 + magic
    nc.vector.tensor_scalar(mask_dve, mask_sb, LOG2E * 128.0, SCHRA_B,
                            ALU.mult, ALU.add)
    run_now(tp_wqk("k", 0, psum_ctx, "ctx"))
    kt_mini(0, psum_ctx, "ctx")
    for sc in range(1, 4):
        run_now(tp_x(sc, psum_ctx, "ctx"))
        kt_mini(sc, psum_ctx, "ctx")
    run_now(tp_wqk("q", 0, psum_ctx, "ctx"))
    run_now(qk_chain("qt", 0, 0, psum_ctx, "ctx"))
    done.update({"qt0s0", "kt0s0"})

    # ---- generators: A gates scores (kt/qt), B gates PV (v) ----
    plan_a = [("x", 4), ("x", 5), ("x", 6), ("x", 7), ("kt", 0, 1),
              ("x", 8), ("x", 9), ("x", 10), ("x", 11), ("kt", 0, 2),
              ("x", 12), ("x", 13), ("x", 14), ("x", 15), ("kt", 0, 3),
              ("qt", 0, 1), ("qt", 0, 2), ("qt", 0, 3)]
    for g in range(1, NG):
        plan_a += [("wgate", g), ("kt", g, 0), ("qt", g, 0), ("kt", g, 1),
                   ("kt", g, 2), ("kt", g, 3), ("qt", g, 1), ("qt", g, 2),
                   ("qt", g, 3)]
    wg_emitted = set()
    plan_b = ([("wv", dc) for dc in range(4)] +
              [("v", sc) for sc in range(SC)])

    def run_plan(plan, pool, tag):
        for item in plan:
            if item[0] == "wgate":
                # stall (empty yields) until the flat loop has emitted the
                # casts + XBAR transposes for this W group
                while item[1] not in wg_emitted:
                    yield
            elif item[0] == "x":
                yield from tp_x(item[1], pool, tag)
                done.add(f"x{item[1]}")
            elif item[0] == "wv":
                yield from tp_wv(item[1], pool, tag)
            elif item[0] == "v":
                sc = item[1]
                if sc >= 4:
                    # xt[sc] comes from gen-A: force its pack to be
                    # emitted first (cross-generator RAW dependency)
                    need(0, f"x{sc}")
                yield from v_chain(sc, pool, tag)
                done.add(f"v{sc}")
            else:
                kind, g, sg = item
                yield from qk_chain(kind, g, sg, pool, tag)
                done.add(f"{kind}{g}s{sg}")

    gens = [run_plan(plan_a, psum_qa, "qa"), run_plan(plan_b, psum_qb, "qb")]

    def drive(n):
        # round-robin both generators
        for _ in range(n):
            alive = [g for g in gens if g is not None]
            if not alive:
                return
            for idx in range(2):
                if gens[idx] is None:
                    continue
                try:
                    next(gens[idx])
                except StopIteration:
                    gens[idx] = None

    def need(idx, *products):
        while gens[idx] is not None and not all(p in done for p in products):
            try:
                next(gens[idx])
            except StopIteration:
                gens[idx] = None

    # ---- attention: flat pipelined stream ----
    blocks = [(g2, qg) for g2 in range(NG) for qg in range(QG)]
    T = len(blocks) * KC
    pend_st = {}
    cur_cp = {}

    def emit_scores(t):
        bi, kc = divmod(t, KC)
        g2, qg = blocks[bi]
        qt, kt = get_qkt("qt", g2), get_qkt("kt", g2)
        ksl = slice(kc * 128, (kc + 1) * 128)
        qsl = slice(qg * 512, (qg + 1) * 512)
        st = psum_mm.tile([128, 2, 512], F32, tag="mm")
        nc.tensor.matmul(st[:, 0, :], kt[0:64, ksl], qt[0:64, qsl],
                         start=True, stop=True)
        nc.tensor.matmul(st[:, 1, :], kt[64:128, ksl], qt[64:128, qsl],
                         start=True, stop=True)
        pend_st[t] = st

    pend_ex = {}
    post = []   # deferred epilogue closures (recip/bcast/mul/dma)

    def emit_exp(t):
        bi, kc = divmod(t, KC)
        st = pend_st.pop(t)
        if _dve_tile(t):
            ex16 = ex_p.tile([128, 2, 512], I16, tag="ex", name="exv")
            nc.vector.tensor_scalar(ex16.rearrange("p a b -> p (a b)"),
                                    st.rearrange("p a b -> p (a b)"),
                                    SCHRA_A, mask_dve[:, kc:kc + 1],
                                    ALU.mult, ALU.add)
            pend_ex[t] = ex16.bitcast(BF16)
        else:
            ex = ex_p.tile([128, 2, 512], BF16, tag="ex", name="exs")
            nc.scalar.activation(ex.rearrange("p a b -> p (a b)"),
                                 st.rearrange("p a b -> p (a b)"),
                                 AF.Exp, bias=mask_sb[:, kc:kc + 1],
                                 scale=1.0 / np.sqrt(DH))
            pend_ex[t] = ex

    def epilogue_a(bi, cp0, cp1):
        g2, qg = blocks[bi]
        qsl = slice(qg * 512, (qg + 1) * 512)
        for h_loc, cp in ((0, cp0), (1, cp1)):
            h = 2 * g2 + h_loc
            ctxs = ctxs_p.tile([DH + 1, 512], F32, tag="ctxs")
            nc.vector.tensor_copy(ctxs, cp)

            def fin(h=h, ctxs=ctxs, qsl=qsl):
                rec = rec_p.tile([1, 512], F32, tag="rec")
                nc.vector.reciprocal_approx_fast(rec, ctxs[0:1, :])
                # ucode library already preloaded by the prologue dummy
                bc = bcs_p.tile([DH + 1, 512], F32, tag="bc")
                nc.gpsimd.partition_broadcast(bc, rec)
                ot = outt_p.tile([DH + 1, 512], F32, tag="outt")
                nc.vector.tensor_mul(ot, ctxs, bc)
                # gpsimd queue: sync is busy with the XBAR W transposes
                nc.gpsimd.dma_start(out=outT[h * DH:(h + 1) * DH, qsl],
                                    in_=ot[1:DH + 1, :])

            post.append(fin)

    for kc in range(AHEAD):
        emit_scores(kc)
    drive(12)
    for t in range(T + LAG):
        if t == 8:
            emit_wg_late(1)
            wg_emitted.add(1)
        elif t == 40:
            emit_wg_late(2)
            wg_emitted.add(2)
        elif t == 76:
            emit_wg_late(3)
            wg_emitted.add(3)
        if t < T:
            emit_exp(t)
            if t + AHEAD < T:
                nbi, nkc = divmod(t + AHEAD, KC)
                ng2, nqg = blocks[nbi]
                need(0, f"kt{ng2}s{nkc // 4}", f"qt{ng2}s{nqg}")
                emit_scores(t + AHEAD)
        pt_ = t - LAG
        if pt_ >= 0:
            pbi, pkc = divmod(pt_, KC)
            pg2, _ = blocks[pbi]
            if pkc == 0:
                cpa = psum_ctx.tile([DH + 1, 512], F32, tag="ctx", name="cpa")
                cpb = psum_ctx.tile([DH + 1, 512], F32, tag="ctx", name="cpb")
                cur_cp[pbi] = (cpa, cpb)
            cp0, cp1 = cur_cp[pbi]
            if pbi == 0:
                need(1, f"v{pkc}")
            ex = pend_ex.pop(pt_)
            nc.tensor.matmul(cp0, v_sb[pkc][:, 2 * pg2, :], ex[:, 0, :],
                             start=(pkc == 0), stop=(pkc == KC - 1))
            nc.tensor.matmul(cp1, v_sb[pkc][:, 2 * pg2 + 1, :], ex[:, 1, :],
                             start=(pkc == 0), stop=(pkc == KC - 1))
            if pkc == KC - 1:
                epilogue_a(pbi, cp0, cp1)
                del cur_cp[pbi]
        if post:
            post.pop(0)()
        drive(2)

    while post:
        post.pop(0)()
    for idx in range(2):
        while gens[idx] is not None:
            try:
                next(gens[idx])
            except StopIteration:
                gens[idx] = None


def build_program():
    nc = bacc.Bacc("TRN2", target_bir_lowering=False, debug=False)
    x = nc.dram_tensor("x", [S, H], F32, kind="ExternalInput").ap()
    mask = nc.dram_tensor("mask", [S], F32, kind="ExternalInput").ap()
    wq = nc.dram_tensor("wq", [DPC, H], F32, kind="ExternalInput").ap()
    bq = nc.dram_tensor("bq", [DPC], F32, kind="ExternalInput").ap()
    wk = nc.dram_tensor("wk", [DPC, H], F32, kind="ExternalInput").ap()
    bk = nc.dram_tensor("bk", [DPC], F32, kind="ExternalInput").ap()
    wv = nc.dram_tensor("wv", [DPC, H], F32, kind="ExternalInput").ap()
    bv = nc.dram_tensor("bv", [DPC], F32, kind="ExternalInput").ap()
    outT = nc.dram_tensor("outT", [DPC, S], F32, kind="ExternalOutput").ap()

    from contextlib import ExitStack
    with tile.TileContext(nc) as tc:
        with ExitStack() as ctx:
            _emit(ctx, tc, nc, x, mask, wq, bq, wk, bk, wv, bv, outT)
    nc.compile()
    return nc


_NC_CACHE = None


def make_in_maps(hidden_states, attention_mask, Wq, bq, Wk, bk, Wv, bv):
    hs = np.asarray(hidden_states, dtype=np.float32)
    am = np.asarray(attention_mask, dtype=np.float32)
    ws = {k: np.asarray(v, dtype=np.float32)
          for k, v in (("wq", Wq), ("bq", bq), ("wk", Wk),
                       ("bk", bk), ("wv", Wv), ("bv", bv))}
    in_maps = []
    for c in range(N_CORES):
        b, g = divmod(c, 2)
        sl = slice(g * DPC, (g + 1) * DPC)
        in_maps.append({
            "x": np.ascontiguousarray(hs[:, b, :]),
            "mask": np.ascontiguousarray(am[b, 0, 0, :]),
            "wq": np.ascontiguousarray(ws["wq"][sl]),
            "bq": np.ascontiguousarray(ws["bq"][sl]),
            "wk": np.ascontiguousarray(ws["wk"][sl]),
            "bk": np.ascontiguousarray(ws["bk"][sl]),
            "wv": np.ascontiguousarray(ws["wv"][sl]),
            "bv": np.ascontiguousarray(ws["bv"][sl]),
        })
    return in_maps


def gather_out(results):
    out = np.empty((S, B, H), np.float32)
    for c in range(N_CORES):
        b, g = divmod(c, 2)
        out[:, b, g * DPC:(g + 1) * DPC] = results[c]["outT"].T
    return out


def kernel(hidden_states, attention_mask, Wq, bq, Wk, bk, Wv, bv):
    global _NC_CACHE
    if _NC_CACHE is None:
        _NC_CACHE = build_program()
    in_maps = make_in_maps(hidden_states, attention_mask,
                           Wq, bq, Wk, bk, Wv, bv)
    res = run_bass_kernel_spmd(_NC_CACHE, in_maps, list(range(N_CORES)))
    return gather_out(res.results)
